# revision 2
# baseline (speedup 1.0000x reference)
"""Self-contained Trainium2 kernel for nn_B3SplineUWT (3-level B3-spline
undecimated wavelet transform), data-parallel over 8 NeuronCores.

kernel(x: [8,1024,1024] f32) -> [8,4,1024,1024] f32  (w1,w2,w3,c3)

Per core: one image, bf16 internal data path (~5e-3 rel, tol 2e-2).
  - H-conv (partition dim): PE banded matmuls, raw integer taps (1,4,6),
    1/256 normalization folded into the ACT PSUM-evacuation scale.
  - W-conv: level d=1 fully on PE via shifted accumulating matmuls
    (stationary w*I, rhs AP offset); levels d=2,4 split PE/DVE, the DVE
    route being 4 chained (1+z^d) bf16 tensor_adds (even shifts keep
    4-byte alignment for the DVE 2x perf mode).
  - subs w_j = c_{j-1} - c_j on DVE in bf16 into bf16 staging.
  - I/O via SWDGE cast-DMAs: f32->bf16 on load, bf16->f32 on store.
"""
import numpy as np

import concourse.bacc as bacc
import concourse.bass as bass
import concourse.mybir as mybir
import concourse.tile as tile
from concourse.bass_utils import run_bass_kernel_spmd

F32 = mybir.dt.float32
BF16 = mybir.dt.bfloat16
ADD = mybir.AluOpType.add
COPY = mybir.ActivationFunctionType.Copy

B = 8
H = 1024
W = 1024
P = 128
NCH = H // P
LEVELS = 3
DILS = (1, 2, 4)
MARG = 16           # left/right margin in yx (>= 2*max(d) = 8, 4B-aligned)
WE = W + 2 * MARG

# chunks routed via PE shifted matmuls per level (rest via DVE cascade)
NPE_PER_LEVEL = (8, 2, 2)

TAPS = {0: 6.0, 1: 4.0, 2: 1.0}   # raw integer taps, exact in bf16
EVAC_SCALE = 1.0 / 256.0          # both 1/16 normalizations, on ACT evac


def _reflect(i, n):
    if i < 0:
        return -i
    if i >= n:
        return 2 * (n - 1) - i
    return i


def _build_h_bands():
    """Banded H-conv matrices (raw taps + reflect), per level, as
    128x128 blocks keyed (chunk_out, chunk_in)."""
    out = []
    for d in DILS:
        full = np.zeros((H, H), np.float32)
        for r in range(H):
            for o in (-2 * d, -d, 0, d, 2 * d):
                full[_reflect(r + o, H), r] += TAPS[abs(o) // d]
        blocks = {}
        for co in range(NCH):
            for ci in range(NCH):
                blk = full[ci * P:(ci + 1) * P, co * P:(co + 1) * P]
                if np.any(blk != 0):
                    blocks[(co, ci)] = np.ascontiguousarray(blk)
        out.append(blocks)
    return out


def _pack_consts(h_bands):
    """Pack deduped 128x128 blocks + scaled identities side by side."""
    mats, seen, index = [], {}, []
    for blocks in h_bands:
        idx = {}
        for key in sorted(blocks):
            b = blocks[key]
            hsh = b.tobytes()
            if hsh not in seen:
                seen[hsh] = len(mats) * P
                mats.append(b)
            idx[key] = seen[hsh]
        index.append(idx)
    ident_offs = {}
    for w in (1.0, 4.0, 6.0):
        ident_offs[w] = len(mats) * P
        mats.append(np.eye(P, dtype=np.float32) * w)
    packed = np.ascontiguousarray(
        np.concatenate(mats, axis=1).astype(np.float32))
    return packed, index, ident_offs


def _build_program():
    h_bands = _build_h_bands()
    consts_np, cindex, ident_offs = _pack_consts(h_bands)
    ncols_const = consts_np.shape[1]

    nc = bacc.Bacc("TRN2", target_bir_lowering=False, debug=False)
    x_d = nc.dram_tensor("x", [H, W], F32, kind="ExternalInput")
    c_d = nc.dram_tensor("consts", [P, ncols_const], F32,
                         kind="ExternalInput")
    out_d = nc.dram_tensor("out", [LEVELS + 1, H, W], F32,
                           kind="ExternalOutput")

    with tile.TileContext(nc) as tc:
        with tc.tile_pool(name="sb", bufs=1) as sb, \
             tc.tile_pool(name="yxp", bufs=3) as yxp, \
             tc.tile_pool(name="casc", bufs=2) as casc, \
             tc.tile_pool(name="wst", bufs=2) as wstp, \
             tc.tile_pool(name="ps", bufs=4, space="PSUM") as ps:

            # constants: f32 DRAM -> bf16 SBUF via SWDGE cast DMA
            cr = sb.tile([P, ncols_const], BF16, tag="cr", name="cr")
            nc.gpsimd.dma_start(cr[:], c_d[:])

            def wident(w):
                off = ident_offs[w]
                return cr[:, off:off + P]

            # input: f32 DRAM -> bf16 SBUF, 4 cast DMAs of 2 chunks each
            xr = sb.tile([P, NCH, W], BF16, tag="xr", name="xr")
            for q in range(4):
                nc.gpsimd.dma_start(
                    xr[:, 2 * q:2 * q + 2, :],
                    bass.AP(x_d, 2 * q * P * W,
                            [[W, P], [P * W, 2], [1, W]]))

            cnr = [sb.tile([P, NCH, W], BF16, tag=f"cnr{i}", name=f"cnr{i}")
                   for i in range(2)]

            for j in range(LEVELS):
                d = DILS[j]
                npe = NPE_PER_LEVEL[j]
                inr = xr if j == 0 else cnr[(j - 1) % 2]
                cur = cnr[j % 2]
                shifts = [(-2 * d, 1.0), (-d, 4.0), (0, 6.0),
                          (d, 4.0), (2 * d, 1.0)]

                wsts = {}
                for co in range(NCH):
                    # ---- H-conv into PSUM (banded matmuls, raw taps) ----
                    pairs = sorted((key, off)
                                   for key, off in cindex[j].items()
                                   if key[0] == co)
                    pt = ps.tile([P, W], F32, tag="psum", name="pt", bufs=4)
                    for half in range(2):
                        for i, ((_, ci), off) in enumerate(pairs):
                            nc.tensor.matmul(
                                pt[:, half * 512:(half + 1) * 512],
                                cr[:, off:off + P],
                                inr[:, ci, half * 512:(half + 1) * 512],
                                start=(i == 0),
                                stop=(i == len(pairs) - 1))

                    # ---- evacuate with 1/256 scale into margin tile ----
                    yx = yxp.tile([P, WE], BF16, tag="yx", name="yx")
                    nc.scalar.activation(yx[:, MARG:MARG + W], pt[:],
                                         COPY, scale=EVAC_SCALE)
                    # reflect margins (Pool): yx[M-k] = yx[M+k]
                    nc.gpsimd.tensor_copy(
                        bass.AP(yx.tensor, MARG - 2 * d, [[WE, P], [1, 2 * d]]),
                        bass.AP(yx.tensor, MARG + 2 * d, [[WE, P], [-1, 2 * d]]))
                    nc.gpsimd.tensor_copy(
                        bass.AP(yx.tensor, MARG + W, [[WE, P], [1, 2 * d]]),
                        bass.AP(yx.tensor, MARG + W - 2, [[WE, P], [-1, 2 * d]]))

                    # ---- W-conv ----
                    if co < npe:
                        # PE route: 5 shifted accumulating matmuls per half
                        pc = ps.tile([P, W], F32, tag="psum", name="pc",
                                     bufs=4)
                        for half in range(2):
                            base = MARG + half * 512
                            for i, (off, wgt) in enumerate(shifts):
                                nc.tensor.matmul(
                                    pc[:, half * 512:(half + 1) * 512],
                                    wident(wgt),
                                    bass.AP(yx.tensor, base + off,
                                            [[WE, P], [1, 512]]),
                                    start=(i == 0),
                                    stop=(i == len(shifts) - 1))
                        nc.scalar.copy(cur[:, co, :], pc[:])
                    else:
                        # DVE route: 4 chained (1 + z^d) adds, bf16 2x mode
                        def yxs(o, width):
                            return bass.AP(yx.tensor, MARG + o,
                                           [[WE, P], [1, width]])
                        t1 = casc.tile([P, WE], BF16, tag="t1", name="t1")
                        t2 = casc.tile([P, WE], BF16, tag="t2", name="t2")
                        w1 = W + 3 * d
                        nc.vector.tensor_add(
                            t1[:, :w1], yxs(-2 * d, w1), yxs(-2 * d + d, w1))
                        w2 = W + 2 * d
                        nc.vector.tensor_add(
                            t2[:, :w2], t1[:, :w2], t1[:, d:d + w2])
                        w3 = W + d
                        nc.vector.tensor_add(
                            t1[:, :w3], t2[:, :w3], t2[:, d:d + w3])
                        nc.vector.tensor_add(
                            cur[:, co, :], t1[:, :W], t1[:, d:d + W])

                    # ---- w_j = prev - cur (bf16) into staging ----
                    hv, ci_ = divmod(co, 4)
                    if ci_ == 0:
                        wsts[hv] = wstp.tile([P, 4, W], BF16, tag="wst",
                                             name="wst")
                    nc.vector.tensor_sub(
                        wsts[hv][:, ci_, :], inr[:, co, :], cur[:, co, :])

                    # ---- stream out per half-plane (cast bf16->f32) ----
                    if ci_ == 3:
                        nc.gpsimd.dma_start(
                            bass.AP(out_d, j * H * W + hv * 4 * P * W,
                                    [[W, P], [P * W, 4], [1, W]]),
                            wsts[hv][:])
                        if j == LEVELS - 1:
                            nc.gpsimd.dma_start(
                                bass.AP(out_d,
                                        3 * H * W + hv * 4 * P * W,
                                        [[W, P], [P * W, 4], [1, W]]),
                                cur[:, hv * 4:(hv + 1) * 4, :])

    nc.compile()
    return nc, consts_np


_CACHE = {}


def _get_program():
    if "prog" not in _CACHE:
        _CACHE["prog"] = _build_program()
    return _CACHE["prog"]


def kernel(x, _trace=False, _trace_kwargs=None):
    """x: [8, 1024, 1024] float32 -> [8, 4, 1024, 1024] float32."""
    x = np.asarray(x)
    assert x.shape == (B, H, W) and x.dtype == np.float32
    nc, consts_np = _get_program()
    in_maps = [{"x": np.ascontiguousarray(x[b]), "consts": consts_np}
               for b in range(B)]
    kw = {}
    if _trace:
        kw = dict(trace=True, **(_trace_kwargs or {}))
    res = run_bass_kernel_spmd(nc, in_maps, core_ids=list(range(B)), **kw)
    out = np.stack([r["out"] for r in res.results], axis=0)
    if _trace:
        return out, res
    return out


# revision 6
# speedup vs baseline: 1.2635x; 1.2635x over previous
"""Self-contained Trainium2 kernel for nn_B3SplineUWT (3-level B3-spline
undecimated wavelet transform), data-parallel over 8 NeuronCores.

kernel(x: [8,1024,1024] f32) -> [8,4,1024,1024] f32  (w1,w2,w3,c3)

Per core: one image, bf16 internal data path (~5e-3 rel, tol 2e-2).
  - H-conv (partition dim): PE banded matmuls, raw integer taps (1,4,6),
    1/256 normalization folded into the ACT PSUM-evacuation scale.
  - W-conv: per-level split between PE (shifted accumulating matmuls,
    stationary w*I, rhs AP offset) and DVE (4 chained (1+z^d) bf16
    tensor_adds; even shifts for d=2,4 keep the DVE 2x perf mode).
  - subs w_j = c_{j-1} - c_j on DVE in bf16 into bf16 staging.
  - I/O via SWDGE cast-DMAs: f32->bf16 on load, bf16->f32 on store.
  - Per-chunk tiles + 1-bank PSUM tiles + deep pools for cross-level
    pipelining.
"""
import numpy as np

import concourse.bacc as bacc
import concourse.bass as bass
import concourse.mybir as mybir
import concourse.tile as tile
from concourse.bass_utils import run_bass_kernel_spmd

F32 = mybir.dt.float32
BF16 = mybir.dt.bfloat16
COPY = mybir.ActivationFunctionType.Copy

B = 8
H = 1024
W = 1024
P = 128
NCH = H // P
LEVELS = 3
DILS = (1, 2, 4)
MARG = 16           # left/right margin in yx (>= 2*max(d) = 8, 4B-aligned)
WE = W + 2 * MARG

# chunks routed via PE shifted matmuls per level (rest via DVE cascade)
NPE_PER_LEVEL = (5, 3, 2)

TAPS = {0: 6.0, 1: 4.0, 2: 1.0}   # raw integer taps, exact in bf16
EVAC_SCALE = 1.0 / 256.0          # both 1/16 normalizations, on ACT evac


def _reflect(i, n):
    if i < 0:
        return -i
    if i >= n:
        return 2 * (n - 1) - i
    return i


def _build_h_bands():
    """Banded H-conv matrices (raw taps + reflect), per level, as
    128x128 blocks keyed (chunk_out, chunk_in)."""
    out = []
    for d in DILS:
        full = np.zeros((H, H), np.float32)
        for r in range(H):
            for o in (-2 * d, -d, 0, d, 2 * d):
                full[_reflect(r + o, H), r] += TAPS[abs(o) // d]
        blocks = {}
        for co in range(NCH):
            for ci in range(NCH):
                blk = full[ci * P:(ci + 1) * P, co * P:(co + 1) * P]
                if np.any(blk != 0):
                    blocks[(co, ci)] = np.ascontiguousarray(blk)
        out.append(blocks)
    return out


def _pack_consts(h_bands):
    """Pack deduped 128x128 blocks + scaled identities side by side."""
    mats, seen, index = [], {}, []
    for blocks in h_bands:
        idx = {}
        for key in sorted(blocks):
            b = blocks[key]
            hsh = b.tobytes()
            if hsh not in seen:
                seen[hsh] = len(mats) * P
                mats.append(b)
            idx[key] = seen[hsh]
        index.append(idx)
    ident_offs = {}
    for w in (1.0, 4.0, 6.0):
        ident_offs[w] = len(mats) * P
        mats.append(np.eye(P, dtype=np.float32) * w)
    packed = np.ascontiguousarray(
        np.concatenate(mats, axis=1).astype(np.float32))
    return packed, index, ident_offs


def _build_program():
    h_bands = _build_h_bands()
    consts_np, cindex, ident_offs = _pack_consts(h_bands)
    ncols_const = consts_np.shape[1]

    nc = bacc.Bacc("TRN2", target_bir_lowering=False, debug=False)
    x_d = nc.dram_tensor("x", [H, W], F32, kind="ExternalInput")
    c_d = nc.dram_tensor("consts", [P, ncols_const], F32,
                         kind="ExternalInput")
    out_d = nc.dram_tensor("out", [LEVELS + 1, H, W], F32,
                           kind="ExternalOutput")

    with tile.TileContext(nc) as tc:
        with tc.tile_pool(name="sb", bufs=1) as sb, \
             tc.tile_pool(name="yxp", bufs=6) as yxp, \
             tc.tile_pool(name="casc", bufs=3) as casc, \
             tc.tile_pool(name="wst", bufs=3) as wstp, \
             tc.tile_pool(name="ps", bufs=8, space="PSUM") as ps:

            # constants: f32 DRAM -> bf16 SBUF via SWDGE cast DMA
            cr = sb.tile([P, ncols_const], BF16, tag="cr", name="cr")
            nc.gpsimd.dma_start(cr[:], c_d[:])

            def wident(w):
                off = ident_offs[w]
                return cr[:, off:off + P]

            # input: f32 DRAM -> bf16 SBUF, 4 cast DMAs of 2 chunks each
            xq = [sb.tile([P, 2, W], BF16, tag=f"xq{q}", name=f"xq{q}")
                  for q in range(4)]
            for q in range(4):
                nc.gpsimd.dma_start(
                    xq[q][:],
                    bass.AP(x_d, 2 * q * P * W,
                            [[W, P], [P * W, 2], [1, W]]))

            # per-chunk level buffers (two generations)
            cbuf = [[sb.tile([P, W], BF16, tag=f"c{g}_{co}",
                             name=f"c{g}_{co}") for co in range(NCH)]
                    for g in range(2)]

            def chunk_in(j, co, lo=0, hi=W):
                if j == 0:
                    return xq[co // 2][:, co % 2, lo:hi]
                return cbuf[(j - 1) % 2][co][:, lo:hi]

            for j in range(LEVELS):
                d = DILS[j]
                npe = NPE_PER_LEVEL[j]
                cur = cbuf[j % 2]
                shifts = [(0, 6.0), (-d, 4.0), (d, 4.0),
                          (-2 * d, 1.0), (2 * d, 1.0)]

                wsts = {}
                for co in range(NCH):
                    # ---- H-conv into PSUM (banded matmuls, raw taps) ----
                    pairs = sorted((key, off)
                                   for key, off in cindex[j].items()
                                   if key[0] == co)
                    yx = yxp.tile([P, WE], BF16, tag="yx", name="yx")
                    for half in range(2):
                        pt = ps.tile([P, 512], F32, tag="psum", name="pt",
                                     bufs=8)
                        for i, ((_, ci), off) in enumerate(pairs):
                            nc.tensor.matmul(
                                pt[:],
                                cr[:, off:off + P],
                                chunk_in(j, ci, half * 512,
                                         (half + 1) * 512),
                                start=(i == 0),
                                stop=(i == len(pairs) - 1))
                        # evacuate with 1/256 scale into margin tile
                        nc.scalar.activation(
                            yx[:, MARG + half * 512:MARG + (half + 1) * 512],
                            pt[:], COPY, scale=EVAC_SCALE)

                    # reflect margins (Pool): yx[M-k] = yx[M+k]
                    nc.gpsimd.tensor_copy(
                        bass.AP(yx.tensor, MARG - 2 * d, [[WE, P], [1, 2 * d]]),
                        bass.AP(yx.tensor, MARG + 2 * d, [[WE, P], [-1, 2 * d]]))
                    nc.gpsimd.tensor_copy(
                        bass.AP(yx.tensor, MARG + W, [[WE, P], [1, 2 * d]]),
                        bass.AP(yx.tensor, MARG + W - 2, [[WE, P], [-1, 2 * d]]))

                    # ---- W-conv ----
                    if co < npe:
                        # PE route: 5 shifted accumulating matmuls per half
                        for half in range(2):
                            pc = ps.tile([P, 512], F32, tag="psum",
                                         name="pc", bufs=8)
                            base = MARG + half * 512
                            for i, (off, wgt) in enumerate(shifts):
                                nc.tensor.matmul(
                                    pc[:],
                                    wident(wgt),
                                    bass.AP(yx.tensor, base + off,
                                            [[WE, P], [1, 512]]),
                                    start=(i == 0),
                                    stop=(i == len(shifts) - 1))
                            nc.scalar.copy(
                                cur[co][:, half * 512:(half + 1) * 512],
                                pc[:])
                    else:
                        # DVE route: 4 chained (1 + z^d) adds, bf16 2x mode
                        def yxs(o, width):
                            return bass.AP(yx.tensor, MARG + o,
                                           [[WE, P], [1, width]])
                        t1 = casc.tile([P, WE], BF16, tag="t1", name="t1")
                        t2 = casc.tile([P, WE], BF16, tag="t2", name="t2")
                        w1 = W + 3 * d
                        nc.vector.tensor_add(
                            t1[:, :w1], yxs(-2 * d, w1), yxs(-d, w1))
                        w2 = W + 2 * d
                        nc.vector.tensor_add(
                            t2[:, :w2], t1[:, :w2], t1[:, d:d + w2])
                        w3 = W + d
                        nc.vector.tensor_add(
                            t1[:, :w3], t2[:, :w3], t2[:, d:d + w3])
                        nc.vector.tensor_add(
                            cur[co][:], t1[:, :W], t1[:, d:d + W])

                    # ---- w_j = prev - cur (bf16) into staging ----
                    hv, ci_ = divmod(co, 4)
                    if ci_ == 0:
                        wsts[hv] = wstp.tile([P, 4, W], BF16, tag="wst",
                                             name="wst")
                    nc.vector.tensor_sub(
                        wsts[hv][:, ci_, :], chunk_in(j, co), cur[co][:])
                    del yx

                    # ---- stream out per half-plane (cast bf16->f32) ----
                    if ci_ == 3:
                        nc.gpsimd.dma_start(
                            bass.AP(out_d, j * H * W + hv * 4 * P * W,
                                    [[W, P], [P * W, 4], [1, W]]),
                            wsts[hv][:])
                        if j == LEVELS - 1:
                            for cc in range(hv * 4, hv * 4 + 4):
                                nc.gpsimd.dma_start(
                                    bass.AP(out_d, 3 * H * W + cc * P * W,
                                            [[W, P], [1, W]]),
                                    cur[cc][:])

    nc.compile()
    return nc, consts_np


_CACHE = {}


def _get_program():
    if "prog" not in _CACHE:
        _CACHE["prog"] = _build_program()
    return _CACHE["prog"]


def kernel(x, _trace=False, _trace_kwargs=None):
    """x: [8, 1024, 1024] float32 -> [8, 4, 1024, 1024] float32."""
    x = np.asarray(x)
    assert x.shape == (B, H, W) and x.dtype == np.float32
    nc, consts_np = _get_program()
    in_maps = [{"x": np.ascontiguousarray(x[b]), "consts": consts_np}
               for b in range(B)]
    kw = {}
    if _trace:
        kw = dict(trace=True, **(_trace_kwargs or {}))
    res = run_bass_kernel_spmd(nc, in_maps, core_ids=list(range(B)), **kw)
    out = np.stack([r["out"] for r in res.results], axis=0)
    if _trace:
        return out, res
    return out


# revision 12
# speedup vs baseline: 1.3050x; 1.0328x over previous
"""Self-contained Trainium2 kernel for nn_B3SplineUWT (3-level B3-spline
undecimated wavelet transform), data-parallel over 8 NeuronCores.

kernel(x: [8,1024,1024] f32) -> [8,4,1024,1024] f32  (w1,w2,w3,c3)

Per core: one image, bf16 internal data path (~5e-3 rel, tol 2e-2).
  - H-conv (partition dim): PE banded matmuls with raw integer taps
    (1,4,6); 2 matmuls per output chunk: a full-K diagonal block plus a
    K=4d boundary block whose input rows are gathered from neighbor
    chunks into partitions 0..4d by small HWDGE SBUF->SBUF DMAs.
    The 1/256 normalization rides the ACT PSUM-evacuation scale.
  - W-conv: per-level split between PE (shifted accumulating matmuls,
    stationary w*I, rhs AP offset) and DVE (4 chained (1+z^d) bf16
    tensor_adds; even shifts for d=2,4 keep the DVE 2x perf mode).
  - subs w_j = c_{j-1} - c_j on DVE in bf16 into bf16 staging.
  - I/O via SWDGE cast-DMAs: f32->bf16 on load, bf16->f32 on store,
    streamed per 2 chunks (and per chunk for c3) to avoid DMA tails.
"""
import numpy as np

import concourse.bacc as bacc
import concourse.bass as bass
import concourse.mybir as mybir
import concourse.tile as tile
from concourse.bass_utils import run_bass_kernel_spmd

F32 = mybir.dt.float32
BF16 = mybir.dt.bfloat16
COPY = mybir.ActivationFunctionType.Copy

B = 8
H = 1024
W = 1024
P = 128
NCH = H // P
LEVELS = 3
DILS = (1, 2, 4)
MARG = 16           # left/right margin in yx (>= 2*max(d) = 8, 4B-aligned)
WE = W + 2 * MARG

# chunks routed via PE shifted matmuls per level (rest via DVE cascade);
# spread so both engines stay busy through each level's tail, with DVE
# getting early chunks (input arrives in chunk order)
PE_ROUTE = ({1, 3, 4, 6, 7}, {1, 3, 5, 7}, {1, 3, 5, 7})

TAPS = {0: 6.0, 1: 4.0, 2: 1.0}   # raw integer taps, exact in bf16
EVAC_SCALE = 1.0 / 256.0          # both 1/16 normalizations, on ACT evac


def _reflect(i, n):
    if i < 0:
        return -i
    if i >= n:
        return 2 * (n - 1) - i
    return i


def _build_blocks():
    """Per level: diagonal 128x128 blocks D[co] (reflect folded at the
    edges) and boundary blocks Bnd[co] whose rows are the neighbor rows
    packed as [above(2d) | below(2d)] (above absent for co=0, below
    absent for co=7)."""
    per_level = []
    for d in DILS:
        full = np.zeros((H, H), np.float32)
        for r in range(H):
            for o in (-2 * d, -d, 0, d, 2 * d):
                full[_reflect(r + o, H), r] += TAPS[abs(o) // d]
        dblk, bblk = [], []
        for co in range(NCH):
            r0 = co * P
            dblk.append(np.ascontiguousarray(full[r0:r0 + P, r0:r0 + P]))
            src = []
            if co > 0:
                src += list(range(r0 - 2 * d, r0))
            if co < NCH - 1:
                src += list(range(r0 + P, r0 + P + 2 * d))
            bb = np.zeros((P, P), np.float32)
            for pp, sr in enumerate(src):
                bb[pp, :] = full[sr, r0:r0 + P]
            bblk.append((np.ascontiguousarray(bb), len(src)))
        per_level.append((dblk, bblk))
    return per_level


def _pack_consts(per_level):
    mats, seen = [], {}

    def intern(m):
        h = m.tobytes()
        if h not in seen:
            seen[h] = len(mats) * P
            mats.append(m)
        return seen[h]

    index = []
    for dblk, bblk in per_level:
        doffs = [intern(m) for m in dblk]
        boffs = [(intern(m), k) for m, k in bblk]
        index.append((doffs, boffs))
    ident_offs = {}
    for w in (1.0, 4.0, 6.0):
        ident_offs[w] = len(mats) * P
        mats.append(np.eye(P, dtype=np.float32) * w)
    packed = np.ascontiguousarray(
        np.concatenate(mats, axis=1).astype(np.float32))
    return packed, index, ident_offs


def _build_program():
    per_level = _build_blocks()
    consts_np, cindex, ident_offs = _pack_consts(per_level)
    ncols_const = consts_np.shape[1]

    nc = bacc.Bacc("TRN2", target_bir_lowering=False, debug=False)
    x_d = nc.dram_tensor("x", [H, W], F32, kind="ExternalInput")
    c_d = nc.dram_tensor("consts", [P, ncols_const], F32,
                         kind="ExternalInput")
    out_d = nc.dram_tensor("out", [LEVELS + 1, H, W], F32,
                           kind="ExternalOutput")

    with tile.TileContext(nc) as tc:
        with tc.tile_pool(name="sb", bufs=1) as sb, \
             tc.tile_pool(name="yxp", bufs=6) as yxp, \
             tc.tile_pool(name="bndp", bufs=6) as bndp, \
             tc.tile_pool(name="casc", bufs=3) as casc, \
             tc.tile_pool(name="wst", bufs=4) as wstp, \
             tc.tile_pool(name="ps", bufs=8, space="PSUM") as ps:

            # constants: f32 DRAM -> bf16 SBUF via SWDGE cast DMA
            cr = sb.tile([P, ncols_const], BF16, tag="cr", name="cr")
            nc.gpsimd.dma_start(cr[:], c_d[:])

            def wident(w):
                off = ident_offs[w]
                return cr[:, off:off + P]

            # input: f32 DRAM -> bf16 SBUF, 4 cast DMAs of 2 chunks each
            xq = [sb.tile([P, 2, W], BF16, tag=f"xq{q}", name=f"xq{q}")
                  for q in range(4)]
            for q in range(4):
                nc.gpsimd.dma_start(
                    xq[q][:],
                    bass.AP(x_d, 2 * q * P * W,
                            [[W, P], [P * W, 2], [1, W]]))

            # per-chunk level buffers (two generations)
            cbuf = [[sb.tile([P, W], BF16, tag=f"c{g}_{co}",
                             name=f"c{g}_{co}") for co in range(NCH)]
                    for g in range(2)]

            def chunk_in(j, co, lo=0, hi=W, p0=0, p1=P):
                if j == 0:
                    return xq[co // 2][p0:p1, co % 2, lo:hi]
                return cbuf[(j - 1) % 2][co][p0:p1, lo:hi]

            for j in range(LEVELS):
                d = DILS[j]
                pe_route = PE_ROUTE[j]
                cur = cbuf[j % 2]
                doffs, boffs = cindex[j]
                shifts = [(0, 6.0), (-d, 4.0), (d, 4.0),
                          (-2 * d, 1.0), (2 * d, 1.0)]

                wsts = {}
                for co in range(NCH):
                    # ---- gather neighbor boundary rows (HWDGE sb->sb) ----
                    boff, bk = boffs[co]
                    bnd = bndp.tile([16, W], BF16, tag="bnd", name="bnd")
                    nb = 0
                    if co > 0:
                        nc.sync.dma_start(
                            bnd[0:2 * d, :],
                            chunk_in(j, co - 1, p0=P - 2 * d, p1=P))
                        nb += 2 * d
                    if co < NCH - 1:
                        nc.sync.dma_start(
                            bnd[nb:nb + 2 * d, :],
                            chunk_in(j, co + 1, p0=0, p1=2 * d))
                        nb += 2 * d
                    assert nb == bk

                    # ---- H-conv into PSUM: diagonal + boundary matmul ----
                    yx = yxp.tile([P, WE], BF16, tag="yx", name="yx")
                    for half in range(2):
                        lo, hi = half * 512, (half + 1) * 512
                        pt = ps.tile([P, 512], F32, tag="psum", name="pt",
                                     bufs=8)
                        nc.tensor.matmul(
                            pt[:], cr[:, doffs[co]:doffs[co] + P],
                            chunk_in(j, co, lo, hi),
                            start=True, stop=False)
                        nc.tensor.matmul(
                            pt[:], cr[0:bk, boff:boff + P],
                            bnd[0:bk, lo:hi],
                            start=False, stop=True)
                        # evacuate with 1/256 scale into margin tile
                        nc.scalar.activation(
                            yx[:, MARG + lo:MARG + hi],
                            pt[:], COPY, scale=EVAC_SCALE)

                    # reflect margins (Pool): yx[M-k] = yx[M+k]
                    nc.gpsimd.tensor_copy(
                        bass.AP(yx.tensor, MARG - 2 * d, [[WE, P], [1, 2 * d]]),
                        bass.AP(yx.tensor, MARG + 2 * d, [[WE, P], [-1, 2 * d]]))
                    nc.gpsimd.tensor_copy(
                        bass.AP(yx.tensor, MARG + W, [[WE, P], [1, 2 * d]]),
                        bass.AP(yx.tensor, MARG + W - 2, [[WE, P], [-1, 2 * d]]))

                    # ---- W-conv ----
                    if co in pe_route:
                        # PE route: 5 shifted accumulating matmuls per half
                        for half in range(2):
                            pc = ps.tile([P, 512], F32, tag="psum",
                                         name="pc", bufs=8)
                            base = MARG + half * 512
                            for i, (off, wgt) in enumerate(shifts):
                                nc.tensor.matmul(
                                    pc[:],
                                    wident(wgt),
                                    bass.AP(yx.tensor, base + off,
                                            [[WE, P], [1, 512]]),
                                    start=(i == 0),
                                    stop=(i == len(shifts) - 1))
                            nc.scalar.copy(
                                cur[co][:, half * 512:(half + 1) * 512],
                                pc[:])
                    else:
                        # DVE route: 4 chained (1 + z^d) adds, bf16 2x mode
                        def yxs(o, width):
                            return bass.AP(yx.tensor, MARG + o,
                                           [[WE, P], [1, width]])
                        t1 = casc.tile([P, WE], BF16, tag="t1", name="t1")
                        t2 = casc.tile([P, WE], BF16, tag="t2", name="t2")
                        w1 = W + 3 * d
                        nc.vector.tensor_add(
                            t1[:, :w1], yxs(-2 * d, w1), yxs(-d, w1))
                        w2 = W + 2 * d
                        nc.vector.tensor_add(
                            t2[:, :w2], t1[:, :w2], t1[:, d:d + w2])
                        w3 = W + d
                        nc.vector.tensor_add(
                            t1[:, :w3], t2[:, :w3], t2[:, d:d + w3])
                        nc.vector.tensor_add(
                            cur[co][:], t1[:, :W], t1[:, d:d + W])

                    # c3: stream each chunk as soon as its W-conv is done
                    if j == LEVELS - 1:
                        nc.gpsimd.dma_start(
                            bass.AP(out_d, 3 * H * W + co * P * W,
                                    [[W, P], [1, W]]),
                            cur[co][:])

                    # ---- w_j = prev - cur (bf16) into staging ----
                    hv, ci_ = divmod(co, 2)
                    if ci_ == 0:
                        wsts[hv] = wstp.tile([P, 2, W], BF16, tag="wst",
                                             name="wst")
                    nc.vector.tensor_sub(
                        wsts[hv][:, ci_, :], chunk_in(j, co), cur[co][:])
                    del yx

                    # ---- stream out per 2 chunks (cast bf16->f32) ----
                    if ci_ == 1:
                        nc.gpsimd.dma_start(
                            bass.AP(out_d, j * H * W + hv * 2 * P * W,
                                    [[W, P], [P * W, 2], [1, W]]),
                            wsts[hv][:])

    nc.compile()
    return nc, consts_np


_CACHE = {}


def _get_program():
    if "prog" not in _CACHE:
        _CACHE["prog"] = _build_program()
    return _CACHE["prog"]


def kernel(x, _trace=False, _trace_kwargs=None):
    """x: [8, 1024, 1024] float32 -> [8, 4, 1024, 1024] float32."""
    x = np.asarray(x)
    assert x.shape == (B, H, W) and x.dtype == np.float32
    nc, consts_np = _get_program()
    in_maps = [{"x": np.ascontiguousarray(x[b]), "consts": consts_np}
               for b in range(B)]
    kw = {}
    if _trace:
        kw = dict(trace=True, **(_trace_kwargs or {}))
    res = run_bass_kernel_spmd(nc, in_maps, core_ids=list(range(B)), **kw)
    out = np.stack([r["out"] for r in res.results], axis=0)
    if _trace:
        return out, res
    return out


# revision 55
# speedup vs baseline: 1.5081x; 1.1557x over previous
"""Self-contained Trainium2 kernel for nn_B3SplineUWT (3-level B3-spline
undecimated wavelet transform), data-parallel over 8 NeuronCores.

kernel(x: [8,1024,1024] f32) -> [8,4,1024,1024] f32  (w1,w2,w3,c3)

Per core: one image, bf16 internal data path (~5e-3 rel, tol 2e-2).
  - H-conv (partition dim): PE banded matmuls with raw integer taps
    (1,4,6), 3-block banded form; the 1/256 normalization (both
    separable passes) rides the ACT PSUM-evacuation scale for free.
  - W-conv: per-level split between PE (5 shifted accumulating matmuls,
    stationary w*I, the shift in the rhs AP offset) and DVE (4 chained
    (1 + z^d) bf16 tensor_adds -- the binomial factorization of the
    5-tap (1,4,6,4,1); even shifts for d=2,4 keep the DVE 2x mode).
  - w_j = c_{j-1} - c_j on DVE in bf16 into bf16 staging.
  - I/O via SWDGE cast-DMAs (f32->bf16 load, bf16->f32 store), streamed
    per 2 chunks (per chunk for c3) so the DMA drains during compute.
  - Per-chunk tiles, one buffer generation per level, 1-bank PSUM
    tiles, wavefront (level,chunk) emission order for cross-level
    pipelining.

Engine budget (cost model, per core): DVE ~52us (critical chain),
DMA ~55us, PE ~47us, ACT ~40us, Pool ~33us; end-to-end ~80us.
"""
import numpy as np

import concourse.bacc as bacc
import concourse.bass as bass
import concourse.mybir as mybir
import concourse.tile as tile
from concourse.bass_utils import run_bass_kernel_spmd

F32 = mybir.dt.float32
BF16 = mybir.dt.bfloat16
COPY = mybir.ActivationFunctionType.Copy

B = 8
H = 1024
W = 1024
P = 128
NCH = H // P
LEVELS = 3
DILS = (1, 2, 4)
MARG = 16           # left/right margin in yx (>= 2*max(d) = 8, 4B-aligned)
WE = W + 2 * MARG

# chunks whose W-conv runs on PE (rest on DVE), per level; tuned so both
# engines stay busy through each level's tail, with DVE getting early
# chunks (input arrives in chunk order)
PE_ROUTE = ({0, 4}, {2, 4, 6}, {2, 4, 6})

YX_BUFS = 6
CASC_BUFS = 3
WST_BUFS = 4
PSUM_BUFS = 8
WAVE_LAG = 3                  # chunk skew between consecutive levels
INPUT_GROUPS = (1, 1, 2, 2, 2)  # chunks per input cast-DMA
W_GROUP = (2, 2, 2)           # chunks per w_j output DMA, per level

TAPS = {0: 6.0, 1: 4.0, 2: 1.0}   # raw integer taps, exact in bf16
EVAC_SCALE = 1.0 / 256.0          # both 1/16 normalizations, on ACT evac


def _reflect(i, n):
    if i < 0:
        return -i
    if i >= n:
        return 2 * (n - 1) - i
    return i


def _build_blocks():
    """Per level: diagonal 128x128 blocks D[co] (reflect folded at the
    edges) and off-diagonal neighbor blocks for the banded H-conv."""
    per_level = []
    for d in DILS:
        full = np.zeros((H, H), np.float32)
        for r in range(H):
            for o in (-2 * d, -d, 0, d, 2 * d):
                full[_reflect(r + o, H), r] += TAPS[abs(o) // d]
        dblk, offdiag = [], []
        for co in range(NCH):
            r0 = co * P
            dblk.append(np.ascontiguousarray(full[r0:r0 + P, r0:r0 + P]))
            od = []
            for ci in (co - 1, co + 1):
                if 0 <= ci < NCH:
                    blk = full[ci * P:(ci + 1) * P, r0:r0 + P]
                    if np.any(blk != 0):
                        od.append((ci, np.ascontiguousarray(blk)))
            offdiag.append(od)
        per_level.append((dblk, offdiag))
    return per_level


def _pack_consts(per_level):
    mats, seen = [], {}

    def intern(m):
        h = m.tobytes()
        if h not in seen:
            seen[h] = len(mats) * P
            mats.append(m)
        return seen[h]

    index = []
    for dblk, offdiag in per_level:
        doffs = [intern(m) for m in dblk]
        ooffs = [[(ci, intern(m)) for ci, m in od] for od in offdiag]
        index.append((doffs, ooffs))
    ident_offs = {}
    for w in (1.0, 4.0, 6.0):
        ident_offs[w] = len(mats) * P
        mats.append(np.eye(P, dtype=np.float32) * w)
    packed = np.ascontiguousarray(
        np.concatenate(mats, axis=1).astype(np.float32))
    return packed, index, ident_offs


def _build_program():
    per_level = _build_blocks()
    consts_np, cindex, ident_offs = _pack_consts(per_level)
    ncols_const = consts_np.shape[1]

    nc = bacc.Bacc("TRN2", target_bir_lowering=False, debug=False)
    x_d = nc.dram_tensor("x", [H, W], F32, kind="ExternalInput")
    c_d = nc.dram_tensor("consts", [P, ncols_const], F32,
                         kind="ExternalInput")
    out_d = nc.dram_tensor("out", [LEVELS + 1, H, W], F32,
                           kind="ExternalOutput")

    with tile.TileContext(nc) as tc:
        with tc.tile_pool(name="sb", bufs=1) as sb, \
             tc.tile_pool(name="yxp", bufs=YX_BUFS) as yxp, \
             tc.tile_pool(name="casc", bufs=CASC_BUFS) as casc, \
             tc.tile_pool(name="wst", bufs=WST_BUFS) as wstp, \
             tc.tile_pool(name="ps", bufs=PSUM_BUFS, space="PSUM") as ps:

            # constants: f32 DRAM -> bf16 SBUF via SWDGE cast DMA
            cr = sb.tile([P, ncols_const], BF16, tag="cr", name="cr")
            nc.gpsimd.dma_start(cr[:], c_d[:])

            def wident(w):
                off = ident_offs[w]
                return cr[:, off:off + P]

            # input: f32 DRAM -> bf16 SBUF cast DMAs; single-chunk loads
            # up front so the first H-convs (and the DVE cascade chain
            # behind them) start as early as possible
            xq = []          # per-chunk accessor: (tile, idx_in_tile)
            for g, n in enumerate(INPUT_GROUPS):
                t = sb.tile([P, n, W], BF16, tag=f"xq{g}", name=f"xq{g}")
                base = sum(INPUT_GROUPS[:g])
                nc.gpsimd.dma_start(
                    t[:],
                    bass.AP(x_d, base * P * W,
                            [[W, P], [P * W, n], [1, W]]))
                for k in range(n):
                    xq.append((t, k))

            # per-chunk level buffers, one generation per level (no WARs)
            cbuf = [[sb.tile([P, W], BF16, tag=f"c{g}_{co}",
                             name=f"c{g}_{co}") for co in range(NCH)]
                    for g in range(LEVELS)]

            def chunk_in(j, co, lo=0, hi=W):
                if j == 0:
                    t, k = xq[co]
                    return t[:, k, lo:hi]
                return cbuf[j - 1][co][:, lo:hi]

            def cur_ap(j, co, lo=0, hi=W):
                return cbuf[j][co][:, lo:hi]

            # wavefront emission order: level j trails level j-1 by
            # WAVE_LAG chunks, so late-level outputs stream from
            # mid-kernel instead of piling into a DMA-only tail
            order = []
            for wave in range(NCH + WAVE_LAG * (LEVELS - 1)):
                for j in range(LEVELS):
                    co = wave - WAVE_LAG * j
                    if 0 <= co < NCH:
                        order.append((j, co))

            wsts = {}
            for j, co in order:
                d = DILS[j]
                doffs, ooffs = cindex[j]
                shifts = [(0, 6.0), (-d, 4.0), (d, 4.0),
                          (-2 * d, 1.0), (2 * d, 1.0)]

                # ---- H-conv into PSUM (banded matmuls, raw taps) ----
                yx = yxp.tile([P, WE], BF16, tag="yx", name="yx")
                for half in range(2):
                    lo, hi = half * 512, (half + 1) * 512
                    pt = ps.tile([P, 512], F32, tag="psum", name="pt",
                                 bufs=PSUM_BUFS)
                    mms = ([(doffs[co], None)] +
                           [(off, ci) for ci, off in ooffs[co]])
                    for i, (off, ci) in enumerate(mms):
                        nc.tensor.matmul(
                            pt[:], cr[:, off:off + P],
                            chunk_in(j, co if ci is None else ci, lo, hi),
                            start=(i == 0),
                            stop=(i == len(mms) - 1))
                    # evacuate with the 1/256 scale into the margin tile
                    nc.scalar.activation(
                        yx[:, MARG + lo:MARG + hi],
                        pt[:], COPY, scale=EVAC_SCALE)

                # reflect margins (Pool): yx[M-k] = yx[M+k]
                nc.gpsimd.tensor_copy(
                    bass.AP(yx.tensor, MARG - 2 * d, [[WE, P], [1, 2 * d]]),
                    bass.AP(yx.tensor, MARG + 2 * d, [[WE, P], [-1, 2 * d]]))
                nc.gpsimd.tensor_copy(
                    bass.AP(yx.tensor, MARG + W, [[WE, P], [1, 2 * d]]),
                    bass.AP(yx.tensor, MARG + W - 2, [[WE, P], [-1, 2 * d]]))

                # ---- W-conv ----
                if co in PE_ROUTE[j]:
                    # PE route: 5 shifted accumulating matmuls per half
                    for half in range(2):
                        pc = ps.tile([P, 512], F32, tag="psum",
                                     name="pc", bufs=PSUM_BUFS)
                        base = MARG + half * 512
                        for i, (off, wgt) in enumerate(shifts):
                            nc.tensor.matmul(
                                pc[:],
                                wident(wgt),
                                bass.AP(yx.tensor, base + off,
                                        [[WE, P], [1, 512]]),
                                start=(i == 0),
                                stop=(i == len(shifts) - 1))
                        nc.scalar.copy(
                            cur_ap(j, co, half * 512, (half + 1) * 512),
                            pc[:])
                else:
                    # DVE route: 4 chained (1 + z^d) adds, bf16 2x mode
                    def yxs(o, width):
                        return bass.AP(yx.tensor, MARG + o,
                                       [[WE, P], [1, width]])
                    t1 = casc.tile([P, WE], BF16, tag="t1", name="t1")
                    t2 = casc.tile([P, WE], BF16, tag="t2", name="t2")
                    w1 = W + 3 * d
                    nc.vector.tensor_add(
                        t1[:, :w1], yxs(-2 * d, w1), yxs(-d, w1))
                    w2 = W + 2 * d
                    nc.vector.tensor_add(
                        t2[:, :w2], t1[:, :w2], t1[:, d:d + w2])
                    w3 = W + d
                    nc.vector.tensor_add(
                        t1[:, :w3], t2[:, :w3], t2[:, d:d + w3])
                    nc.vector.tensor_add(
                        cur_ap(j, co), t1[:, :W], t1[:, d:d + W])

                # c3: stream each chunk as soon as its W-conv is done
                if j == LEVELS - 1:
                    nc.gpsimd.dma_start(
                        bass.AP(out_d, 3 * H * W + co * P * W,
                                [[W, P], [1, W]]),
                        cur_ap(j, co))

                # ---- w_j = prev - cur (bf16) into staging ----
                wg = W_GROUP[j]
                hv, ci_ = divmod(co, wg)
                if ci_ == 0:
                    wsts[(j, hv)] = wstp.tile([P, wg, W], BF16,
                                              tag="wst", name="wst")
                nc.vector.tensor_sub(
                    wsts[(j, hv)][:, ci_, :], chunk_in(j, co),
                    cur_ap(j, co))

                # ---- stream out per group (cast bf16->f32) ----
                if ci_ == wg - 1:
                    nc.gpsimd.dma_start(
                        bass.AP(out_d, j * H * W + hv * wg * P * W,
                                [[W, P], [P * W, wg], [1, W]]),
                        wsts[(j, hv)][:])

    nc.compile()
    return nc, consts_np


_CACHE = {}


def _get_program():
    if "prog" not in _CACHE:
        _CACHE["prog"] = _build_program()
    return _CACHE["prog"]


def kernel(x, _trace=False, _trace_kwargs=None):
    """x: [8, 1024, 1024] float32 -> [8, 4, 1024, 1024] float32."""
    x = np.asarray(x)
    assert x.shape == (B, H, W) and x.dtype == np.float32
    nc, consts_np = _get_program()
    in_maps = [{"x": np.ascontiguousarray(x[b]), "consts": consts_np}
               for b in range(B)]
    kw = {}
    if _trace:
        kw = dict(trace=True, **(_trace_kwargs or {}))
    res = run_bass_kernel_spmd(nc, in_maps, core_ids=list(range(B)), **kw)
    out = np.stack([r["out"] for r in res.results], axis=0)
    if _trace:
        return out, res
    return out
